# revision 10
# baseline (speedup 1.0000x reference)
"""Trainium2 Bass kernel for MeshGenLoss (Chamfer + KL + density-uniformity).

Math: d[i,j] = |a_i|^2 + |b_j|^2 - 2 a_i.b_j as ONE K=33 bf16 matmul per
[128,512] tile (3 exact bf16 limbs per fp32 scalar -> fp32-exact distances
in PSUM at bf16 matmul speed).

v3 structure: only TWO matrices are computed (pred->target "pt" and the
self matrix "pp"); the transposed direction is never materialized.
 - pt row mins -> pred-side Chamfer term (DVE wide reduce-min per block).
 - pt COLUMN mins -> target-side Chamfer term: per-core column-min slab
   built by chained TT-mins (2x bf16), xbar DMA-transposed, then one 3D
   free-axis reduce -> per-core partial col-mins, min-combined on host.
 - pp is SYMMETRIC, so its row mins == its column mins: pp jobs get NO
   row reduction at all, only the (cheap) column-min machinery. Host
   un-rotates and min-combines across cores.

Per core: 16 jobs of [128 rows x 4096 cols] (8 pt + 8 pp), each as 2 PSUM
chunks [128,2048] (pool bufs=2 = all 8 banks). ACT evacuates chunks to
bf16 st tiles; DVE does the mins. pp diagonal mask: extra matmul
(1000*I)^T @ (1000*I) accumulated into the chunk adds 1e6 on the
(host-rotated) diagonal. Job order staggers pt/pp so each batch's col-min
chain (merge TTs -> DMA transpose -> reduce) overlaps the next stretch of
matmuls instead of trailing at the end.

Sharding: core c owns pred/target rows [512c, 512c+512). pp columns are
host-rotated by -512c so the diagonal falls in chunk 0 at offset 128r
(identical SPMD program on all cores).
"""

import sys

import ml_dtypes
import numpy as np

sys.path.insert(0, "/opt/trn_rl_repo")

B = 2
N = 4096
L = 512
CORES = 8
ROWS = N // CORES  # 512 rows per core
RB = ROWS // 128  # 4 row blocks per core
K = 33
BF16 = ml_dtypes.bfloat16

# per-batch job order: b0 pt-early (rhs_p DMA trails), b1 pp-early (so the
# last colmin chain overlaps the trailing pt stretch)
JOB_ORDER = {
    0: [("pt", 0), ("pt", 1), ("pp", 0), ("pt", 2), ("pp", 1),
        ("pt", 3), ("pp", 2), ("pp", 3)],
    1: [("pt", 0), ("pp", 0), ("pt", 1), ("pp", 1),
        ("pt", 2), ("pt", 3), ("pp", 2), ("pp", 3)],
}


def _limbs3(x):
    """Split float64 array into 3 bf16 limbs capturing ~24 significand bits."""
    h = x.astype(BF16)
    r = x - h.astype(np.float64)
    m = r.astype(BF16)
    r2 = r - m.astype(np.float64)
    lo = r2.astype(BF16)
    return h, m, lo


def _build_lhsT(a):
    """a: [n, 3] float64 row points -> lhsT [33, n] bf16."""
    n = a.shape[0]
    asq = (a * a).sum(-1)
    al = _limbs3(a)
    sl = _limbs3(asq)
    out = np.zeros((K, n), dtype=BF16)
    k = 0
    for t in range(3):
        for p in range(3):
            row = (-2.0 * al[p][:, t].astype(np.float64)).astype(BF16)
            for _q in range(3):
                out[k] = row
                k += 1
    for p in range(3):
        out[k] = sl[p]
        k += 1
    for _q in range(3):
        out[k] = np.ones(n, dtype=BF16)
        k += 1
    return out


def _build_rhs(b):
    """b: [m, 3] float64 column points -> rhs [33, m] bf16."""
    m = b.shape[0]
    bsq = (b * b).sum(-1)
    bl = _limbs3(b)
    sl = _limbs3(bsq)
    out = np.zeros((K, m), dtype=BF16)
    k = 0
    for t in range(3):
        for _p in range(3):
            for q in range(3):
                out[k] = bl[q][:, t]
                k += 1
    for _p in range(3):
        out[k] = np.ones(m, dtype=BF16)
        k += 1
    for q in range(3):
        out[k] = sl[q]
        k += 1
    return out


def _build_program():
    import concourse.bacc as bacc
    import concourse.mybir as mybir
    import concourse.tile as tile
    from contextlib import ExitStack

    dt = mybir.dt
    Alu = mybir.AluOpType
    Act = mybir.ActivationFunctionType

    nc = bacc.Bacc("TRN2", target_bir_lowering=False, debug=False)

    d_lhsT = nc.declare_dram_parameter("lhsT", [B, K, ROWS], dt.bfloat16, isOutput=False)
    d_rhs_t = nc.declare_dram_parameter("rhs_t", [B, K, N], dt.bfloat16, isOutput=False)
    d_rhs_p = nc.declare_dram_parameter("rhs_p", [B, K, N], dt.bfloat16, isOutput=False)
    d_dql = nc.declare_dram_parameter("dql", [128, 128], dt.bfloat16, isOutput=False)
    d_mu = nc.declare_dram_parameter("mu_sl", [1, 128], dt.float32, isOutput=False)
    d_lv = nc.declare_dram_parameter("lv_sl", [1, 128], dt.float32, isOutput=False)

    # o_min: pt rowmins, col = 4*b + r
    o_min = nc.declare_dram_parameter("o_min", [128, 8], dt.float32, isOutput=True)
    # o_ct: transposed colmins packed [128, (2*ki+b)*32 + t]; ki 0=pt, 1=pp
    o_ct = nc.declare_dram_parameter("o_ct", [128, 128], dt.float32, isOutput=True)
    o_kl = nc.declare_dram_parameter("o_kl", [1, 3], dt.float32, isOutput=True)

    with tile.TileContext(nc) as tc, ExitStack() as ctx:
        consts = ctx.enter_context(tc.tile_pool(name="consts", bufs=1))
        psum = ctx.enter_context(tc.tile_pool(name="psum", bufs=2, space="PSUM"))
        stpool = ctx.enter_context(tc.tile_pool(name="st", bufs=6))
        slabpool = ctx.enter_context(tc.tile_pool(name="slab", bufs=3))
        utpool = ctx.enter_context(tc.tile_pool(name="ut", bufs=2))

        # ---- resident inputs --------------------------------------------
        lhsT_sb = {}
        rhs_sb = {}
        for b in range(B):
            t1 = consts.tile([K, ROWS], dt.bfloat16, tag=f"l{b}")
            rt = consts.tile([K, N], dt.bfloat16, tag=f"rt{b}")
            rp = consts.tile([K, N], dt.bfloat16, tag=f"rp{b}")
            lhsT_sb[b] = t1
            rhs_sb["pt", b] = rt
            rhs_sb["pp", b] = rp
        # critical path: lhsT r0 slice + first rhs cols, on separate queues
        nc.sync.dma_start(out=lhsT_sb[0][:, :128], in_=d_lhsT[0, :, :128])
        nc.scalar.dma_start(out=rhs_sb["pt", 0][:, :512], in_=d_rhs_t[0, :, :512])
        nc.sync.dma_start(out=lhsT_sb[0][:, 128:], in_=d_lhsT[0, :, 128:])
        nc.scalar.dma_start(out=rhs_sb["pt", 0][:, 512:1024], in_=d_rhs_t[0, :, 512:1024])
        nc.scalar.dma_start(out=rhs_sb["pt", 0][:, 1024:2048], in_=d_rhs_t[0, :, 1024:2048])
        nc.sync.dma_start(out=rhs_sb["pt", 0][:, 2048:], in_=d_rhs_t[0, :, 2048:])
        nc.gpsimd.dma_start(out=rhs_sb["pp", 0][:, :2048], in_=d_rhs_p[0, :, :2048])
        nc.gpsimd.dma_start(out=rhs_sb["pp", 0][:, 2048:], in_=d_rhs_p[0, :, 2048:])
        dql_sb = consts.tile([128, 128], dt.bfloat16, tag="dql")
        nc.scalar.dma_start(out=dql_sb[:], in_=d_dql[:])
        nc.sync.dma_start(out=lhsT_sb[1][:], in_=d_lhsT[1])
        # b1: pp first in job order, so rhs_p[1] before rhs_t[1]
        nc.gpsimd.dma_start(out=rhs_sb["pp", 1][:, :2048], in_=d_rhs_p[1, :, :2048])
        nc.gpsimd.dma_start(out=rhs_sb["pp", 1][:, 2048:], in_=d_rhs_p[1, :, 2048:])
        nc.sync.dma_start(out=rhs_sb["pt", 1][:, :2048], in_=d_rhs_t[1, :, :2048])
        nc.sync.dma_start(out=rhs_sb["pt", 1][:, 2048:], in_=d_rhs_t[1, :, 2048:])
        mu_sb = consts.tile([1, 128], dt.float32, tag="mu")
        nc.scalar.dma_start(out=mu_sb[:], in_=d_mu[:])
        lv_sb = consts.tile([1, 128], dt.float32, tag="lv")
        nc.scalar.dma_start(out=lv_sb[:], in_=d_lv[:])

        omin_sb = consts.tile([128, 8], dt.float32, tag="omin")
        ctall = consts.tile([128, 128], dt.float32, tag="ctall")
        ct_off = {("pt", 0): 0, ("pt", 1): 32, ("pp", 0): 64, ("pp", 1): 96}

        def make_chunk(lhsT, rhs, h, diag_r=None):
            """One [128,2048] PSUM chunk = 4 matmuls; optional diagonal add."""
            ch = psum.tile([128, 2048], dt.float32, tag="ps")
            for t in range(4):
                c0 = 2048 * h + 512 * t
                nc.tensor.matmul(
                    ch[:, 512 * t:512 * (t + 1)], lhsT, rhs[:, c0:c0 + 512],
                    start=True, stop=not (diag_r is not None and t == 0),
                )
            if diag_r is not None:
                nc.tensor.matmul(
                    ch[:, 128 * diag_r:128 * diag_r + 128], dql_sb[:], dql_sb[:],
                    start=False, stop=True, skip_group_check=True,
                )
            return ch

        # per (kind, batch) colmin slab with progressive merge
        slabs = {}
        nmerged = {}

        def merge_into_slab(kind, b, st):
            if (kind, b) not in slabs:
                slabs[kind, b] = st  # first job's st doubles as slab seed
                nmerged[kind, b] = 1
                return
            if nmerged[kind, b] == 1:
                sl = slabpool.tile([128, 4096], dt.bfloat16, tag="sl")
                nc.vector.tensor_tensor(sl[:], slabs[kind, b][:], st[:], Alu.min)
                slabs[kind, b] = sl
            else:
                sl = slabs[kind, b]
                if nmerged[kind, b] == RB - 1:
                    nc.vector.tensor_tensor(
                        sl[:, :2048], sl[:, :2048], st[:, :2048], Alu.min)
                    nc.vector.tensor_tensor(
                        sl[:, 2048:], sl[:, 2048:], st[:, 2048:], Alu.min)
                else:
                    nc.vector.tensor_tensor(sl[:], sl[:], st[:], Alu.min)
            nmerged[kind, b] += 1
            if nmerged[kind, b] == RB:
                sl = slabs[kind, b]
                ut = utpool.tile([128, 32, 128], dt.bfloat16, tag="ut")
                nc.sync.dma_start_transpose(ut[:], sl[:])
                ctb = utpool.tile([128, 32], dt.bfloat16, tag="ctb")
                nc.vector.tensor_reduce(
                    ctb[:], ut[:], axis=mybir.AxisListType.X, op=Alu.min)
                o = ct_off[kind, b]
                nc.scalar.copy(ctall[:, o:o + 32], ctb[:])

        for b in range(B):
            for kind, r in JOB_ORDER[b]:
                lhsT = lhsT_sb[b][:, 128 * r:128 * (r + 1)]
                rhs = rhs_sb[kind, b]
                st = stpool.tile([128, 4096], dt.bfloat16, tag="st")
                c0 = make_chunk(lhsT, rhs, 0, diag_r=r if kind == "pp" else None)
                nc.scalar.copy(st[:, :2048], c0[:])
                c1 = make_chunk(lhsT, rhs, 1)
                nc.scalar.copy(st[:, 2048:], c1[:])
                if kind == "pt":
                    nc.vector.tensor_reduce(
                        omin_sb[:, 4 * b + r:4 * b + r + 1], st[:],
                        axis=mybir.AxisListType.X, op=Alu.min)
                merge_into_slab(kind, b, st)

        # ---- KL partials ------------------------------------------------
        klt = consts.tile([1, 3], dt.float32, tag="klt")
        nc.vector.tensor_reduce(klt[:, 0:1], lv_sb[:], axis=mybir.AxisListType.X, op=Alu.add)
        e_t = consts.tile([1, 128], dt.float32, tag="klexp")
        nc.scalar.activation(e_t[:], lv_sb[:], Act.Exp, accum_out=klt[:, 2:3])
        sq_t = consts.tile([1, 128], dt.float32, tag="klsq")
        nc.scalar.activation(sq_t[:], mu_sb[:], Act.Square, accum_out=klt[:, 1:2])

        # ---- outputs ----------------------------------------------------
        nc.sync.dma_start(out=o_min[:], in_=omin_sb[:])
        nc.sync.dma_start(out=o_ct[:], in_=ctall[:])
        nc.sync.dma_start(out=o_kl[:], in_=klt[:])

    nc.compile()
    return nc


def _make_in_maps(pred, target, mu, logvar):
    pred = np.asarray(pred, dtype=np.float32)
    target = np.asarray(target, dtype=np.float32)
    mu = np.asarray(mu, dtype=np.float32)
    logvar = np.asarray(logvar, dtype=np.float32)

    pred64 = pred.astype(np.float64)
    target64 = target.astype(np.float64)

    rhs_t = np.stack([_build_rhs(target64[b]) for b in range(B)])  # [B,K,N]
    rhs_p_full = np.stack([_build_rhs(pred64[b]) for b in range(B)])
    dql = (np.eye(128) * 1000.0).astype(BF16)
    mu_flat = mu.reshape(-1)
    lv_flat = logvar.reshape(-1)

    in_maps = []
    for c in range(CORES):
        rows = slice(ROWS * c, ROWS * (c + 1))
        lhsT = np.stack([_build_lhsT(pred64[b, rows]) for b in range(B)])
        rot = np.roll(rhs_p_full, -ROWS * c, axis=2)
        in_maps.append({
            "lhsT": lhsT,
            "rhs_t": rhs_t,
            "rhs_p": np.ascontiguousarray(rot),
            "dql": dql,
            "mu_sl": mu_flat[128 * c:128 * (c + 1)].reshape(1, 128),
            "lv_sl": lv_flat[128 * c:128 * (c + 1)].reshape(1, 128),
        })
    return in_maps


def kernel(pred, target, mu, logvar):
    from concourse.bass_utils import run_bass_kernel_spmd

    in_maps = _make_in_maps(pred, target, mu, logvar)
    nc = _build_program()
    res = run_bass_kernel_spmd(nc, in_maps, list(range(CORES)))
    results = res.results

    # pt rowmins
    nn_pt = np.empty((B, N), dtype=np.float64)
    for c in range(CORES):
        om = results[c]["o_min"].astype(np.float64)  # [128, 8]
        for b in range(B):
            for r in range(RB):
                rows = slice(ROWS * c + 128 * r, ROWS * c + 128 * r + 128)
                nn_pt[b, rows] = om[:, 4 * b + r]

    # colmins: o_ct[c][j_rel, (2*ki+b)*32 + t] = min over core c's rows
    cts = np.stack([r["o_ct"] for r in results]).astype(np.float64)
    cts = cts.reshape(CORES, 128, 2, B, 32).transpose(0, 2, 3, 1, 4)
    # [CORES, 2, B, 128, 32] -> per-core col vector [CORES, 2, B, 4096]
    colv = cts.transpose(0, 1, 2, 4, 3).reshape(CORES, 2, B, N)
    nn_tp = colv[:, 0].min(axis=0)  # [B, N]
    # pp: un-rotate each core's columns (core c col j' = global (j'+512c)%N)
    pp_parts = np.full((CORES, B, N), np.inf)
    for c in range(CORES):
        pp_parts[c] = np.roll(colv[c, 1], ROWS * c, axis=1)
    nn_pp = pp_parts.min(axis=0)  # [B, N]

    kl_parts = np.stack([r["o_kl"].reshape(3) for r in results])

    cd = (nn_pt.mean(axis=1) + nn_tp.mean(axis=1)).mean()

    s1 = kl_parts[:, 0].astype(np.float64).sum()
    s2 = kl_parts[:, 1].astype(np.float64).sum()
    s3 = kl_parts[:, 2].astype(np.float64).sum()
    n_kl = B * L
    kl = -0.5 * (n_kl + s1 - s2 - s3) / n_kl

    density = np.std(nn_pp, axis=1, ddof=1).mean()

    total = cd + 0.001 * kl + 0.1 * density

    return (
        np.float32(total),
        np.float32(cd),
        np.float32(kl),
        np.float32(density),
    )
